# revision 12
# baseline (speedup 1.0000x reference)
"""Multi-head attention forward (B=2, S=2048, D=1024, H=16) on 8 TRN2 cores.

Sharding: hybrid tensor/data parallel. Cores 0-3 take batch 0, cores 4-7
batch 1; within a batch each core owns 4 heads (256 of 1024 features).
The host pre-transposes activations/weights, folds the 1/sqrt(dk) scale
into Wq/bq and the v-bias into the output bias, and sums the 4 partial
output projections per batch at the end.

Per-core dataflow (feature-on-partition for q/k, token-on-partition for v):
  qT/kT    = W @ X.T          (PE; ACT identity applies bias, writes f16)
  v        = X @ Wv.T         (PE, natural layout; DVE copies to f16 + ones col)
  sT       = kT.T @ qT        (PE; 4-slot PSUM ring, 2 heads per kt)
  eT       = exp(sT)          (ACT; no max-subtraction: scores ~ N(0,1))
  ctxT     = v_aug.T @ eT     (PE; 65th lhsT column accumulates denominators)
  norm     = DVE row-copy -> recip_approx_fast -> gpsimd partition_broadcast
             -> DVE mult (no PE, no PSUM)
  out      = ctxT.T @ WoT     (PE, deferred into later blocks' k-loops,
                               borrowing the idle cx PSUM buffer)
"""

import sys
import types

import numpy as np

# ---------------------------------------------------------------------------
# Problem constants (hardcoded; kernel.py must be self-contained)
# ---------------------------------------------------------------------------
B = 2  # batch
S = 2048  # sequence length
D = 1024  # model dim
H = 16  # heads
DK = D // H  # 64 head dim
NCORES = 8
CPB = NCORES // B  # cores per batch = 4
FH = D // CPB  # features per core = 256 (4 heads)
P = 128
KD = D // P  # 8 contraction k-tiles for projections
KT = S // P  # 16 key-token tiles
NM = FH // P  # 2 m-tiles per core = head pairs
QS = 512  # q-slice width for the attention inner loop
NQS = S // QS  # 4
NEG_SCALE = 1.0 / np.sqrt(DK)  # folded into Wq/bq on the host


def _install_ntff_hook():
    """Recreate antenv.axon_hooks so trace=True can profile via axon."""
    if "antenv.axon_hooks" in sys.modules:
        return
    try:
        import antenv
    except ImportError:
        return
    mod = types.ModuleType("antenv.axon_hooks")
    mod._hook = None
    mod.set_axon_ntff_profile_hook = lambda h: setattr(mod, "_hook", h)
    mod.get_axon_ntff_profile_hook = lambda: mod._hook
    sys.modules["antenv.axon_hooks"] = mod
    antenv.axon_hooks = mod
    try:
        from trn_agent_boot.trn_boot import _ntff_profile_via_ctypes

        mod.set_axon_ntff_profile_hook(
            _ntff_profile_via_ctypes("/opt/axon/libaxon_pjrt.so")
        )
    except Exception:
        pass


_NC_CACHE = {}


def _build_nc(debug=False):
    """Build the per-core Bass program (identical on all 8 cores)."""
    from contextlib import ExitStack

    import concourse.bass as bass  # noqa: F401
    import concourse.mybir as mybir
    import concourse.tile as tile
    from concourse import bacc

    f32 = mybir.dt.float32
    f16 = mybir.dt.float16
    AF = mybir.ActivationFunctionType

    nc = bacc.Bacc()

    xtq = nc.dram_tensor("xtq", [D, S], f16, kind="ExternalInput")
    xtk = nc.dram_tensor("xtk", [D, S], f16, kind="ExternalInput")
    xtv = nc.dram_tensor("xtv", [D, S], f16, kind="ExternalInput")
    wqt = nc.dram_tensor("wqt", [D, FH], f16, kind="ExternalInput")
    wkt = nc.dram_tensor("wkt", [D, FH], f16, kind="ExternalInput")
    wvt = nc.dram_tensor("wvt", [D, FH], f16, kind="ExternalInput")
    wot = nc.dram_tensor("wot", [FH, D], f16, kind="ExternalInput")
    bqd = nc.dram_tensor("bqd", [P, NM], f32, kind="ExternalInput")
    bkd = nc.dram_tensor("bkd", [P, NM], f32, kind="ExternalInput")
    out = nc.dram_tensor("out", [S, D], f16, kind="ExternalOutput")
    if debug:
        dbg_qt = nc.dram_tensor("dbg_qt", [P, NM, S], f16, kind="ExternalOutput")
        dbg_kt = nc.dram_tensor("dbg_kt", [P, NM, S], f16, kind="ExternalOutput")
        dbg_va = nc.dram_tensor(
            "dbg_va", [P, KT, 4, DK + 1], f16, kind="ExternalOutput"
        )
        dbg_rc = nc.dram_tensor(
            "dbg_rc", [NM * NQS, 2 * QS], f32, kind="ExternalOutput"
        )
        dbg_ctx = nc.dram_tensor("dbg_ctx", [P, NM, S], f16, kind="ExternalOutput")

    with tile.TileContext(nc) as tc, ExitStack() as ctx:
        const = ctx.enter_context(tc.tile_pool(name="const", bufs=1))
        wpool = ctx.enter_context(tc.tile_pool(name="wpool", bufs=1))
        xpool = ctx.enter_context(tc.tile_pool(name="xpool", bufs=1))
        persist = ctx.enter_context(tc.tile_pool(name="persist", bufs=1))
        expool = ctx.enter_context(tc.tile_pool(name="expool", bufs=4))
        dnpool = ctx.enter_context(tc.tile_pool(name="dnpool", bufs=2))
        rcpool = ctx.enter_context(tc.tile_pool(name="rcpool", bufs=2))
        bcpool = ctx.enter_context(tc.tile_pool(name="bcpool", bufs=2))
        obpool = ctx.enter_context(tc.tile_pool(name="obpool", bufs=4))

        # --- weights + biases (sync sequencer), activations (pool/vector
        # sequencers) — parallel issue streams, consumption order ---
        wq_sb = wpool.tile([P, KD, FH], f16)
        wk_sb = wpool.tile([P, KD, FH], f16)
        wv_sb = wpool.tile([P, KD, FH], f16)
        wo_sb = wpool.tile([P, NM, D], f16)
        bq_sb = const.tile([P, NM], f32)
        bk_sb = const.tile([P, NM], f32)
        xq_sb = xpool.tile([P, KD, S], f16)
        xk_sb = xpool.tile([P, KD, S], f16)
        xv_sb = xpool.tile([P, KD, S], f16)

        nc.sync.dma_start(wq_sb, wqt[:, :].rearrange("(ko p) f -> p ko f", p=P))
        for ko in range(KD):
            nc.gpsimd.dma_start(xq_sb[:, ko, :], xtq[ko * P : (ko + 1) * P, :])
        nc.sync.dma_start(bq_sb, bqd[:, :])
        nc.sync.dma_start(wk_sb, wkt[:, :].rearrange("(ko p) f -> p ko f", p=P))
        for ko in range(KD):
            nc.gpsimd.dma_start(xk_sb[:, ko, :], xtk[ko * P : (ko + 1) * P, :])
        nc.sync.dma_start(bk_sb, bkd[:, :])
        nc.sync.dma_start(wv_sb, wvt[:, :].rearrange("(ko p) f -> p ko f", p=P))
        for ko in range(KD):
            nc.gpsimd.dma_start(xv_sb[:, ko, :], xtv[ko * P : (ko + 1) * P, :])
        nc.sync.dma_start(wo_sb, wot[:, :].rearrange("(m p) d -> p m d", p=P))

        # --- persistent activations ---
        qt_sb = persist.tile([P, NM, S], f16)
        kt_sb = persist.tile([P, NM, S], f16)
        ctx_sb = persist.tile([P, NM, S], f16)
        vaug = persist.tile([P, KT, 4, DK + 1], f16)
        nc.vector.memset(vaug[:, :, :, DK : DK + 1], 1.0)
        vaug4 = vaug.rearrange("p t h x -> p (t h) x")

        # ------------------------------------------------------------------
        # Phase 1: projections.
        #   q/k: feature-on-partition, ACT identity applies bias -> f16.
        #   v:   token-on-partition (natural), DVE copy -> vaug (+ones col).
        # ------------------------------------------------------------------
        with (
            tc.tile_pool(name="pp", bufs=2, space="PSUM") as pp,
            tc.tile_pool(name="vpp", bufs=2, space="PSUM") as vpp,
        ):

            def proj_mk(xsb, w_sb, b_sb, dst, m):
                for hf in range(2):
                    ps = pp.tile([P, 1024], f32, tag="pp", name=f"ps{m}_{hf}")
                    for ko in range(KD):
                        for ns in range(2):
                            nc.tensor.matmul(
                                ps[:, ns * 512 : (ns + 1) * 512],
                                lhsT=w_sb[:, ko, m * P : (m + 1) * P],
                                rhs=xsb[
                                    :,
                                    ko,
                                    hf * 1024 + ns * 512 : hf * 1024 + (ns + 1) * 512,
                                ],
                                start=(ko == 0),
                                stop=(ko == KD - 1),
                            )
                    nc.scalar.activation(
                        dst[:, m, hf * 1024 : (hf + 1) * 1024],
                        ps,
                        AF.Identity,
                        bias=b_sb[:, m : m + 1],
                    )

            proj_mk(xq_sb, wq_sb, bq_sb, qt_sb, 0)
            proj_mk(xk_sb, wk_sb, bk_sb, kt_sb, 0)

            for st in range(KT):
                vps = vpp.tile([P, FH], f32, tag="vp", name=f"vps{st}")
                for ko in range(KD):
                    nc.tensor.matmul(
                        vps,
                        lhsT=xv_sb[:, ko, st * P : (st + 1) * P],
                        rhs=wv_sb[:, ko, :],
                        start=(ko == 0),
                        stop=(ko == KD - 1),
                    )
                nc.vector.tensor_copy(
                    vaug[:, st, :, 0:DK],
                    vps.rearrange("p (h x) -> p h x", x=DK),
                )

            proj_mk(xq_sb, wq_sb, bq_sb, qt_sb, 1)
            proj_mk(xk_sb, wk_sb, bk_sb, kt_sb, 1)

        # ------------------------------------------------------------------
        # Phase 2: attention. Blocks = (q-slice, pair); 16 kt iterations of
        # scoresT -> exp -> PV per block, PV skewed 2 kt behind. Scores live
        # in a manual 4-slot PSUM ring (aligned pairs per kt, full-kt PE
        # lookahead). ctx accumulates in two explicit 2-bank cx tiles that
        # alternate per block; deferred out-proj chunks borrow the idle one.
        # Normalization is a pure DVE/GpSimd dataflow chain.
        # ------------------------------------------------------------------
        with (
            tc.tile_pool(name="scp", bufs=1, space="PSUM") as scp_pool,
            tc.tile_pool(name="cxp", bufs=1, space="PSUM") as cxp,
        ):
            scp = scp_pool.tile([P, 4 * 512], f32)
            cxt = [cxp.tile([P, 2 * QS], f32, name=f"cx{i}") for i in range(2)]
            pending = []

            def out_chunk(mt, ns, po):
                for pair in range(NM):
                    nc.tensor.matmul(
                        po,
                        lhsT=ctx_sb[:, pair, mt * P : (mt + 1) * P],
                        rhs=wo_sb[:, pair, ns * 512 : (ns + 1) * 512],
                        start=(pair == 0),
                        stop=(pair == NM - 1),
                        skip_group_check=True,
                    )
                ob = obpool.tile([P, 512], f16, tag="ob")
                nc.vector.tensor_copy(ob, po)
                nc.sync.dma_start(
                    out[mt * P : (mt + 1) * P, ns * 512 : (ns + 1) * 512], ob
                )

            for blk in range(NQS * NM):
                qs, pair = divmod(blk, NM)
                q0 = qs * QS
                cx = cxt[blk % 2]
                spare = cxt[1 - blk % 2]
                nchunk = 0
                exq = []
                for kt in range(KT):
                    base = 2 * (kt % 2)  # slots (0,1) / (2,3)
                    for h in range(2):
                        nc.tensor.matmul(
                            scp[:, (base + h) * 512 : (base + h + 1) * 512],
                            lhsT=kt_sb[
                                64 * h : 64 * (h + 1), pair, kt * P : (kt + 1) * P
                            ],
                            rhs=qt_sb[64 * h : 64 * (h + 1), pair, q0 : q0 + QS],
                            start=True,
                            stop=True,
                        )
                    ex = expool.tile([P, 2 * QS], f16, tag="ex")
                    nc.scalar.activation(
                        ex, scp[:, base * 512 : (base + 2) * 512], AF.Exp
                    )
                    exq.append((kt, ex))
                    if kt >= 5 and kt % 2 == 1 and pending and nchunk < 4:
                        # out-proj chunk borrows half of the idle cx buffer
                        pending.pop(0)(spare[:, (nchunk % 2) * 512 : (nchunk % 2 + 1) * 512])
                        nchunk += 1
                    if len(exq) > 2:
                        _pv(nc, cx, vaug4, exq.pop(0), pair)
                for item in exq:
                    _pv(nc, cx, vaug4, item, pair)

                # normalization: pure DVE/GpSimd chain, no PE involvement
                dn = dnpool.tile([1, 2 * QS], f32, tag="dn")
                nc.vector.tensor_copy(dn, cx[DK : DK + 1, :])
                rcp = rcpool.tile([1, 2 * QS], f32, tag="rcp")
                nc.vector.reciprocal_approx_fast(rcp, dn)
                bc = bcpool.tile([DK, 2 * QS], f32, tag="bc")
                nc.gpsimd.partition_broadcast(bc, rcp, channels=DK)
                for h in range(2):
                    nc.vector.tensor_mul(
                        ctx_sb[64 * h : 64 * (h + 1), pair, q0 : q0 + QS],
                        cx[0:DK, h * QS : (h + 1) * QS],
                        bc[:, h * QS : (h + 1) * QS],
                    )
                if debug:
                    b_ = pair * NQS + qs
                    nc.sync.dma_start(dbg_rc[b_ : b_ + 1, :], rcp)
                if pair == NM - 1:
                    for sub in range(QS // P):
                        mt = qs * (QS // P) + sub
                        for ns in range(2):
                            pending.append(lambda po, m=mt, n=ns: out_chunk(m, n, po))

        # tail: drain remaining out-proj chunks with a fresh 4-deep pool
        with tc.tile_pool(name="tpo", bufs=4, space="PSUM") as tpo:
            for i, fn in enumerate(pending):
                tp = tpo.tile([P, 512], f32, tag="tpo", name=f"tpo{i}")
                fn(tp)

        if debug:
            nc.sync.dma_start(dbg_qt[:, :, :], qt_sb[:, :, :])
            nc.sync.dma_start(dbg_kt[:, :, :], kt_sb[:, :, :])
            nc.sync.dma_start(dbg_va[:, :, :, :], vaug[:, :, :, :])
            nc.sync.dma_start(dbg_ctx[:, :, :], ctx_sb[:, :, :])

    nc.finalize()
    return nc


def _pv(nc, cx, vaug4, item, pair):
    kt, ex = item
    for h in range(2):
        nc.tensor.matmul(
            cx[0 : DK + 1, h * QS : (h + 1) * QS],
            lhsT=vaug4[:, kt * 4 + 2 * pair + h, :],
            rhs=ex[:, h * QS : (h + 1) * QS],
            start=(kt == 0),
            stop=(kt == KT - 1),
            skip_group_check=True,
        )


def _get_nc():
    if "nc" not in _NC_CACHE:
        _install_ntff_hook()
        _NC_CACHE["nc"] = _build_nc()
    return _NC_CACHE["nc"]


def _make_in_maps(query, key, value, Wq, bq, Wk, bk, Wv, bv, Wo):
    qn = np.asarray(query, np.float32)
    kn = np.asarray(key, np.float32)
    vn = np.asarray(value, np.float32)
    Wq = np.asarray(Wq, np.float32)
    Wk = np.asarray(Wk, np.float32)
    Wv = np.asarray(Wv, np.float32)
    Wo = np.asarray(Wo, np.float32)
    bq = np.asarray(bq, np.float32)
    bk = np.asarray(bk, np.float32)

    xt = {}
    for b in range(B):
        xt[b] = (
            np.ascontiguousarray(qn[b].T).astype(np.float16),
            np.ascontiguousarray(kn[b].T).astype(np.float16),
            np.ascontiguousarray(vn[b].T).astype(np.float16),
        )

    in_maps = []
    for c in range(NCORES):
        b, hp = divmod(c, CPB)
        sl = slice(hp * FH, (hp + 1) * FH)
        in_maps.append(
            {
                "xtq": xt[b][0],
                "xtk": xt[b][1],
                "xtv": xt[b][2],
                "wqt": np.ascontiguousarray((Wq[sl, :] * NEG_SCALE).T).astype(
                    np.float16
                ),
                "wkt": np.ascontiguousarray(Wk[sl, :].T).astype(np.float16),
                "wvt": np.ascontiguousarray(Wv[sl, :].T).astype(np.float16),
                "wot": np.ascontiguousarray(Wo[:, sl].T).astype(np.float16),
                "bqd": np.ascontiguousarray((bq[sl] * NEG_SCALE).reshape(NM, P).T),
                "bkd": np.ascontiguousarray(bk[sl].reshape(NM, P).T),
            }
        )
    return in_maps


def _run(inputs, trace=False):
    from concourse.bass_utils import run_bass_kernel_spmd

    nc = _get_nc()
    in_maps = _make_in_maps(
        inputs["query"],
        inputs["key"],
        inputs["value"],
        inputs["Wq"],
        inputs["bq"],
        inputs["Wk"],
        inputs["bk"],
        inputs["Wv"],
        inputs["bv"],
        inputs["Wo"],
    )
    res = run_bass_kernel_spmd(nc, in_maps, list(range(NCORES)), trace=trace)
    bo = np.asarray(inputs["bo"], np.float32)
    bv = np.asarray(inputs["bv"], np.float32)
    Wo = np.asarray(inputs["Wo"], np.float32)
    obias = bo + bv @ Wo.T  # v-bias contributes bv @ Wo.T exactly (attn sums to 1)
    out = np.zeros((B, S, D), np.float32)
    for c in range(NCORES):
        out[c // CPB] += res.results[c]["out"].astype(np.float32)
    out += obias[None, None, :]
    return out, res


def kernel(**inputs) -> np.ndarray:
    out, _ = _run(inputs, trace=False)
    return out


# revision 14
# speedup vs baseline: 1.3920x; 1.3920x over previous
"""Multi-head attention forward (B=2, S=2048, D=1024, H=16) on 8 TRN2 cores.

Sharding: hybrid tensor/data parallel. Cores 0-3 take batch 0, cores 4-7
batch 1; within a batch each core owns 4 heads (256 of 1024 features).
The host pre-transposes activations/weights, folds the 1/sqrt(dk) scale
into Wq/bq and the v-bias into the output bias, and sums the 4 partial
output projections per batch at the end.

Per-core dataflow (feature-on-partition for q/k, token-on-partition for v):
  qT/kT    = W @ X.T          (PE; ACT identity applies bias, writes f16)
  v        = X @ Wv.T         (PE, natural layout; DVE copies to f16 + ones col)
  sT       = kT.T @ qT        (PE; 4-slot PSUM ring, 2 heads per kt)
  eT       = exp(sT)          (ACT; no max-subtraction: scores ~ N(0,1))
  ctxT     = v_aug.T @ eT     (PE; 65th lhsT column accumulates denominators)
  norm     = DVE row-copy -> recip_approx_fast -> gpsimd partition_broadcast
             -> DVE mult (no PE, no PSUM)
  out      = ctxT.T @ WoT     (PE, deferred into later blocks' k-loops,
                               borrowing the idle cx PSUM buffer)
"""

import sys
import types

import numpy as np

# ---------------------------------------------------------------------------
# Problem constants (hardcoded; kernel.py must be self-contained)
# ---------------------------------------------------------------------------
B = 2  # batch
S = 2048  # sequence length
D = 1024  # model dim
H = 16  # heads
DK = D // H  # 64 head dim
NCORES = 8
CPB = NCORES // B  # cores per batch = 4
FH = D // CPB  # features per core = 256 (4 heads)
P = 128
KD = D // P  # 8 contraction k-tiles for projections
KT = S // P  # 16 key-token tiles
NM = FH // P  # 2 m-tiles per core = head pairs
QS = 512  # q-slice width for the attention inner loop
NQS = S // QS  # 4
NEG_SCALE = 1.0 / np.sqrt(DK)  # folded into Wq/bq on the host


def _install_ntff_hook():
    """Recreate antenv.axon_hooks so trace=True can profile via axon."""
    if "antenv.axon_hooks" in sys.modules:
        return
    try:
        import antenv
    except ImportError:
        return
    mod = types.ModuleType("antenv.axon_hooks")
    mod._hook = None
    mod.set_axon_ntff_profile_hook = lambda h: setattr(mod, "_hook", h)
    mod.get_axon_ntff_profile_hook = lambda: mod._hook
    sys.modules["antenv.axon_hooks"] = mod
    antenv.axon_hooks = mod
    try:
        from trn_agent_boot.trn_boot import _ntff_profile_via_ctypes

        mod.set_axon_ntff_profile_hook(
            _ntff_profile_via_ctypes("/opt/axon/libaxon_pjrt.so")
        )
    except Exception:
        pass


_NC_CACHE = {}


def _build_nc(debug=False):
    """Build the per-core Bass program (identical on all 8 cores)."""
    from contextlib import ExitStack

    import concourse.bass as bass  # noqa: F401
    import concourse.mybir as mybir
    import concourse.tile as tile
    from concourse import bacc

    f32 = mybir.dt.float32
    f16 = mybir.dt.float16
    AF = mybir.ActivationFunctionType

    nc = bacc.Bacc()

    xtq = nc.dram_tensor("xtq", [D, S], f16, kind="ExternalInput")
    xtk = nc.dram_tensor("xtk", [D, S], f16, kind="ExternalInput")
    xtv = nc.dram_tensor("xtv", [D, S], f16, kind="ExternalInput")
    wqt = nc.dram_tensor("wqt", [D, FH], f16, kind="ExternalInput")
    wkt = nc.dram_tensor("wkt", [D, FH], f16, kind="ExternalInput")
    wvt = nc.dram_tensor("wvt", [D, FH], f16, kind="ExternalInput")
    wot = nc.dram_tensor("wot", [FH, D], f16, kind="ExternalInput")
    bqd = nc.dram_tensor("bqd", [P, NM], f32, kind="ExternalInput")
    bkd = nc.dram_tensor("bkd", [P, NM], f32, kind="ExternalInput")
    out = nc.dram_tensor("out", [S, D], f16, kind="ExternalOutput")
    if debug:
        dbg_qt = nc.dram_tensor("dbg_qt", [P, NM, S], f16, kind="ExternalOutput")
        dbg_kt = nc.dram_tensor("dbg_kt", [P, NM, S], f16, kind="ExternalOutput")
        dbg_va = nc.dram_tensor(
            "dbg_va", [P, KT, 4, DK + 1], f16, kind="ExternalOutput"
        )
        dbg_rc = nc.dram_tensor(
            "dbg_rc", [NM * NQS, 2 * QS], f32, kind="ExternalOutput"
        )
        dbg_ctx = nc.dram_tensor("dbg_ctx", [P, NM, S], f16, kind="ExternalOutput")

    with tile.TileContext(nc) as tc, ExitStack() as ctx:
        const = ctx.enter_context(tc.tile_pool(name="const", bufs=1))
        wpool = ctx.enter_context(tc.tile_pool(name="wpool", bufs=1))
        xpool = ctx.enter_context(tc.tile_pool(name="xpool", bufs=1))
        persist = ctx.enter_context(tc.tile_pool(name="persist", bufs=1))
        expool = ctx.enter_context(tc.tile_pool(name="expool", bufs=4))
        dnpool = ctx.enter_context(tc.tile_pool(name="dnpool", bufs=2))
        rcpool = ctx.enter_context(tc.tile_pool(name="rcpool", bufs=2))
        bcpool = ctx.enter_context(tc.tile_pool(name="bcpool", bufs=2))
        obpool = ctx.enter_context(tc.tile_pool(name="obpool", bufs=4))

        # --- weights + biases (sync sequencer), activations (pool/vector
        # sequencers) — parallel issue streams, consumption order ---
        wq_sb = wpool.tile([P, KD, FH], f16)
        wk_sb = wpool.tile([P, KD, FH], f16)
        wv_sb = wpool.tile([P, KD, FH], f16)
        wo_sb = wpool.tile([P, NM, D], f16)
        bq_sb = const.tile([P, NM], f32)
        bk_sb = const.tile([P, NM], f32)
        xq_sb = xpool.tile([P, KD, S], f16)
        xk_sb = xpool.tile([P, KD, S], f16)
        xv_sb = xpool.tile([P, KD, S], f16)

        nc.sync.dma_start(wq_sb, wqt[:, :].rearrange("(ko p) f -> p ko f", p=P))
        for ko in range(KD):
            nc.gpsimd.dma_start(xq_sb[:, ko, :], xtq[ko * P : (ko + 1) * P, :])
        nc.sync.dma_start(bq_sb, bqd[:, :])
        nc.sync.dma_start(wk_sb, wkt[:, :].rearrange("(ko p) f -> p ko f", p=P))
        for ko in range(KD):
            nc.gpsimd.dma_start(xk_sb[:, ko, :], xtk[ko * P : (ko + 1) * P, :])
        nc.sync.dma_start(bk_sb, bkd[:, :])
        nc.sync.dma_start(wv_sb, wvt[:, :].rearrange("(ko p) f -> p ko f", p=P))
        for ko in range(KD):
            nc.gpsimd.dma_start(xv_sb[:, ko, :], xtv[ko * P : (ko + 1) * P, :])
        nc.sync.dma_start(wo_sb, wot[:, :].rearrange("(m p) d -> p m d", p=P))

        # --- persistent activations ---
        qt_sb = persist.tile([P, NM, S], f16)
        kt_sb = persist.tile([P, NM, S], f16)
        ctx_sb = persist.tile([P, NM, S], f16)
        vaug = persist.tile([P, KT, 4, DK + 1], f16)
        nc.vector.memset(vaug[:, :, :, DK : DK + 1], 1.0)
        vaug4 = vaug.rearrange("p t h x -> p (t h) x")

        # ------------------------------------------------------------------
        # Phase 1: projections.
        #   q/k: feature-on-partition, ACT identity applies bias -> f16.
        #   v:   token-on-partition (natural), DVE copy -> vaug (+ones col).
        # ------------------------------------------------------------------
        with (
            tc.tile_pool(name="pp", bufs=2, space="PSUM") as pp,
            tc.tile_pool(name="vpp", bufs=2, space="PSUM") as vpp,
        ):

            def proj_mk(xsb, w_sb, b_sb, dst, m):
                for hf in range(2):
                    ps = pp.tile([P, 1024], f32, tag="pp", name=f"ps{m}_{hf}")
                    for ko in range(KD):
                        for ns in range(2):
                            nc.tensor.matmul(
                                ps[:, ns * 512 : (ns + 1) * 512],
                                lhsT=w_sb[:, ko, m * P : (m + 1) * P],
                                rhs=xsb[
                                    :,
                                    ko,
                                    hf * 1024 + ns * 512 : hf * 1024 + (ns + 1) * 512,
                                ],
                                start=(ko == 0),
                                stop=(ko == KD - 1),
                            )
                    nc.scalar.activation(
                        dst[:, m, hf * 1024 : (hf + 1) * 1024],
                        ps,
                        AF.Identity,
                        bias=b_sb[:, m : m + 1],
                    )

            proj_mk(xq_sb, wq_sb, bq_sb, qt_sb, 0)
            proj_mk(xk_sb, wk_sb, bk_sb, kt_sb, 0)

            for st in range(KT):
                vps = vpp.tile([P, FH], f32, tag="vp", name=f"vps{st}")
                for ko in range(KD):
                    nc.tensor.matmul(
                        vps,
                        lhsT=xv_sb[:, ko, st * P : (st + 1) * P],
                        rhs=wv_sb[:, ko, :],
                        start=(ko == 0),
                        stop=(ko == KD - 1),
                    )
                nc.vector.tensor_copy(
                    vaug[:, st, :, 0:DK],
                    vps.rearrange("p (h x) -> p h x", x=DK),
                )

            proj_mk(xq_sb, wq_sb, bq_sb, qt_sb, 1)
            proj_mk(xk_sb, wk_sb, bk_sb, kt_sb, 1)

        # ------------------------------------------------------------------
        # Phase 2: attention. Blocks = (q-slice, pair); 16 kt iterations of
        # scoresT -> exp -> PV per block, PV skewed 2 kt behind. Scores live
        # in a manual 4-slot PSUM ring (aligned pairs per kt, full-kt PE
        # lookahead). ctx accumulates in two explicit 2-bank cx tiles that
        # alternate per block; deferred out-proj chunks borrow the idle one.
        # Normalization is a pure DVE/GpSimd dataflow chain.
        # ------------------------------------------------------------------
        with (
            tc.tile_pool(name="scp", bufs=2, space="PSUM") as scp_pool,
            tc.tile_pool(name="cxp", bufs=1, space="PSUM") as cxp,
        ):
            cxt = [cxp.tile([P, 2 * QS], f32, name=f"cx{i}") for i in range(2)]
            pending = []

            def out_chunk(mt, ns, po):
                for pair in range(NM):
                    nc.tensor.matmul(
                        po,
                        lhsT=ctx_sb[:, pair, mt * P : (mt + 1) * P],
                        rhs=wo_sb[:, pair, ns * 512 : (ns + 1) * 512],
                        start=(pair == 0),
                        stop=(pair == NM - 1),
                        skip_group_check=True,
                    )
                ob = obpool.tile([P, 512], f16, tag="ob")
                nc.vector.tensor_copy(ob, po)
                nc.sync.dma_start(
                    out[mt * P : (mt + 1) * P, ns * 512 : (ns + 1) * 512], ob
                )

            for blk in range(NQS * NM):
                qs, pair = divmod(blk, NM)
                q0 = qs * QS
                cx = cxt[blk % 2]
                spare = cxt[1 - blk % 2]
                nchunk = 0
                exq = []
                for kt in range(KT):
                    sc = scp_pool.tile([P, 2 * QS], f32, tag="sc")
                    for h in range(2):
                        nc.tensor.matmul(
                            sc[:, h * 512 : (h + 1) * 512],
                            lhsT=kt_sb[
                                64 * h : 64 * (h + 1), pair, kt * P : (kt + 1) * P
                            ],
                            rhs=qt_sb[64 * h : 64 * (h + 1), pair, q0 : q0 + QS],
                            start=True,
                            stop=True,
                        )
                    ex = expool.tile([P, 2 * QS], f16, tag="ex")
                    nc.scalar.activation(ex, sc, AF.Exp)
                    exq.append((kt, ex))
                    if kt >= 5 and kt % 2 == 1 and pending and nchunk < 4:
                        # out-proj chunk borrows half of the idle cx buffer
                        pending.pop(0)(spare[:, (nchunk % 2) * 512 : (nchunk % 2 + 1) * 512])
                        nchunk += 1
                    if len(exq) > 2:
                        _pv(nc, cx, vaug4, exq.pop(0), pair)
                for item in exq:
                    _pv(nc, cx, vaug4, item, pair)

                # normalization: pure DVE/GpSimd chain, no PE involvement
                dn = dnpool.tile([1, 2 * QS], f32, tag="dn")
                nc.vector.tensor_copy(dn, cx[DK : DK + 1, :])
                rcp = rcpool.tile([1, 2 * QS], f32, tag="rcp")
                nc.vector.reciprocal_approx_fast(rcp, dn)
                bc = bcpool.tile([DK, 2 * QS], f32, tag="bc")
                nc.gpsimd.partition_broadcast(bc, rcp, channels=DK)
                for h in range(2):
                    nc.vector.tensor_mul(
                        ctx_sb[64 * h : 64 * (h + 1), pair, q0 : q0 + QS],
                        cx[0:DK, h * QS : (h + 1) * QS],
                        bc[:, h * QS : (h + 1) * QS],
                    )
                if debug:
                    b_ = pair * NQS + qs
                    nc.sync.dma_start(dbg_rc[b_ : b_ + 1, :], rcp)
                if pair == NM - 1:
                    for sub in range(QS // P):
                        mt = qs * (QS // P) + sub
                        for ns in range(2):
                            pending.append(lambda po, m=mt, n=ns: out_chunk(m, n, po))

        # tail: drain remaining out-proj chunks with a fresh 4-deep pool
        with tc.tile_pool(name="tpo", bufs=4, space="PSUM") as tpo:
            for i, fn in enumerate(pending):
                tp = tpo.tile([P, 512], f32, tag="tpo", name=f"tpo{i}")
                fn(tp)

        if debug:
            nc.sync.dma_start(dbg_qt[:, :, :], qt_sb[:, :, :])
            nc.sync.dma_start(dbg_kt[:, :, :], kt_sb[:, :, :])
            nc.sync.dma_start(dbg_va[:, :, :, :], vaug[:, :, :, :])
            nc.sync.dma_start(dbg_ctx[:, :, :], ctx_sb[:, :, :])

    nc.finalize()
    return nc


def _pv(nc, cx, vaug4, item, pair):
    kt, ex = item
    for h in range(2):
        nc.tensor.matmul(
            cx[0 : DK + 1, h * QS : (h + 1) * QS],
            lhsT=vaug4[:, kt * 4 + 2 * pair + h, :],
            rhs=ex[:, h * QS : (h + 1) * QS],
            start=(kt == 0),
            stop=(kt == KT - 1),
            skip_group_check=True,
        )


def _get_nc():
    if "nc" not in _NC_CACHE:
        _install_ntff_hook()
        _NC_CACHE["nc"] = _build_nc()
    return _NC_CACHE["nc"]


def _make_in_maps(query, key, value, Wq, bq, Wk, bk, Wv, bv, Wo):
    qn = np.asarray(query, np.float32)
    kn = np.asarray(key, np.float32)
    vn = np.asarray(value, np.float32)
    Wq = np.asarray(Wq, np.float32)
    Wk = np.asarray(Wk, np.float32)
    Wv = np.asarray(Wv, np.float32)
    Wo = np.asarray(Wo, np.float32)
    bq = np.asarray(bq, np.float32)
    bk = np.asarray(bk, np.float32)

    xt = {}
    for b in range(B):
        xt[b] = (
            np.ascontiguousarray(qn[b].T).astype(np.float16),
            np.ascontiguousarray(kn[b].T).astype(np.float16),
            np.ascontiguousarray(vn[b].T).astype(np.float16),
        )

    in_maps = []
    for c in range(NCORES):
        b, hp = divmod(c, CPB)
        sl = slice(hp * FH, (hp + 1) * FH)
        in_maps.append(
            {
                "xtq": xt[b][0],
                "xtk": xt[b][1],
                "xtv": xt[b][2],
                "wqt": np.ascontiguousarray((Wq[sl, :] * NEG_SCALE).T).astype(
                    np.float16
                ),
                "wkt": np.ascontiguousarray(Wk[sl, :].T).astype(np.float16),
                "wvt": np.ascontiguousarray(Wv[sl, :].T).astype(np.float16),
                "wot": np.ascontiguousarray(Wo[:, sl].T).astype(np.float16),
                "bqd": np.ascontiguousarray((bq[sl] * NEG_SCALE).reshape(NM, P).T),
                "bkd": np.ascontiguousarray(bk[sl].reshape(NM, P).T),
            }
        )
    return in_maps


def _run(inputs, trace=False):
    from concourse.bass_utils import run_bass_kernel_spmd

    nc = _get_nc()
    in_maps = _make_in_maps(
        inputs["query"],
        inputs["key"],
        inputs["value"],
        inputs["Wq"],
        inputs["bq"],
        inputs["Wk"],
        inputs["bk"],
        inputs["Wv"],
        inputs["bv"],
        inputs["Wo"],
    )
    res = run_bass_kernel_spmd(nc, in_maps, list(range(NCORES)), trace=trace)
    bo = np.asarray(inputs["bo"], np.float32)
    bv = np.asarray(inputs["bv"], np.float32)
    Wo = np.asarray(inputs["Wo"], np.float32)
    obias = bo + bv @ Wo.T  # v-bias contributes bv @ Wo.T exactly (attn sums to 1)
    out = np.zeros((B, S, D), np.float32)
    for c in range(NCORES):
        out[c // CPB] += res.results[c]["out"].astype(np.float32)
    out += obias[None, None, :]
    return out, res


def kernel(**inputs) -> np.ndarray:
    out, _ = _run(inputs, trace=False)
    return out
